# revision 12
# baseline (speedup 1.0000x reference)
"""RBF similarity: out[b, n] = exp(-gamma * ||inputs[b] - sample_matrix[n]||^2).

Strategy (8 trn2 NeuronCores, data-parallel over query rows):
  - Shard B=8192 query rows into 8 shards of 1024; replicate sample_matrix.
  - GEMM trick: -gamma*||x-s||^2 = 2g*x.s - g*||x||^2 - g*||s||^2.
  - Device computes psum = x_bf16.T @ s_bf16 augmented with 2 extra
    contraction rows that carry -0.5*||s||^2 (split hi/lo in bf16 so the
    norm term keeps ~fp32 precision), then one ScalarE activation per
    4-bank PSUM half evicts it as exp(2g*psum - g*||x||^2), using the
    per-partition bias input for the -g*||x||^2 term.
  - Raw bass (manual semaphores): the walrus build here allows at most
    one sync-wait per instruction, which Tile's scheduler exceeds.
  - Host does layout prep only: transpose to contraction-major, bf16
    cast, and the (negligible-FLOP) row norms.

Pipeline per core: PE fills one 4-bank PSUM half (12 matmuls) while ACT
evicts the other; output rows ping-pong between two SBUF row-tiles whose
DMA-out overlaps the next row's compute.
"""

import numpy as np
import ml_dtypes

import concourse.bass as bass
import concourse.mybir as mybir
from concourse.bass import ts
from concourse.bass_utils import run_bass_kernel_spmd

GAMMA = 0.001
B, D, N = 8192, 256, 4096
NCORES = 8
B_LOC = B // NCORES          # 1024 query rows per core
M_TILES = B_LOC // 128       # 8 PSUM-partition tiles
KTAIL = 32                   # augmented tail k-tile: rows 0/1 carry -0.5*||s||^2 hi/lo
NB = 512                     # matmul free dim = one PSUM bank (fp32)
HALF = 2048                  # 4 banks per PSUM half
HALVES = 2 * M_TILES         # 16 half-iterations

BF16 = mybir.dt.bfloat16
F32 = mybir.dt.float32
U16 = mybir.dt.uint16
OUT_SCALE = 65535.0  # device writes round(out * 65535) as uint16; host rescales


def _build() -> bass.Bass:
    nc = bass.Bass(name="rbf_similarity", trn_type="TRN2")
    xT = nc.dram_tensor("xt", [D, B_LOC], BF16, kind="ExternalInput")
    sT = nc.dram_tensor("st", [D, N], BF16, kind="ExternalInput")
    xTl = nc.dram_tensor("xtail", [128, B_LOC], BF16, kind="ExternalInput")
    sTl = nc.dram_tensor("stail", [128, N], BF16, kind="ExternalInput")
    xsq = nc.dram_tensor("xsq", [128, M_TILES], F32, kind="ExternalInput")
    out = nc.dram_tensor("out", [B_LOC, N], U16, kind="ExternalOutput")

    with (
        nc.sbuf_tensor([128, B_LOC], BF16) as x0,
        nc.sbuf_tensor([128, B_LOC], BF16) as x1,
        nc.sbuf_tensor([128, B_LOC], BF16) as x2,
        nc.sbuf_tensor([128, N], BF16) as s0,
        nc.sbuf_tensor([128, N], BF16) as s1,
        nc.sbuf_tensor([128, N], BF16) as s2,
        nc.sbuf_tensor([128, M_TILES], F32) as xq,
        nc.sbuf_tensor([128, 1], F32) as scratch,
        nc.sbuf_tensor([128, N], U16) as ot0,
        nc.sbuf_tensor([128, N], U16) as ot1,
        nc.psum_tensor([128, HALF], F32) as psA,
        nc.psum_tensor([128, HALF], F32) as psB,
        nc.semaphore("k0_sem") as k0_sem,
        nc.semaphore("k1_sem") as k1_sem,
        nc.semaphore("k2_sem") as k2_sem,
        nc.semaphore("pe_sem") as pe_sem,
        nc.semaphore("act_sem") as act_sem,
        nc.semaphore("od_sem") as od_sem,
        nc.Block() as block,
    ):
        xs = [x0, x1, x2]
        ss = [s0, s1, s2]
        ots = [ot0, ot1]
        pss = [psA, psB]

        @block.sync
        def _(sync):
            # per-k-group semaphores: the PE starts after 1.25 MB (k0 group)
            # instead of the full 4.7 MB input load
            sync.dma_start(x0[:], xT[0:128, :]).then_inc(k0_sem, 16)
            sync.dma_start(s0[:], sT[0:128, :]).then_inc(k0_sem, 16)
            sync.dma_start(x1[:], xT[128:256, :]).then_inc(k1_sem, 16)
            sync.dma_start(s1[:], sT[128:256, :]).then_inc(k1_sem, 16)
            sync.dma_start(x2[:], xTl[:, :]).then_inc(k2_sem, 16)
            sync.dma_start(s2[:], sTl[:, :]).then_inc(k2_sem, 16)
            sync.dma_start(xq[:], xsq[:, :]).then_inc(k2_sem, 16)

        @block.tensor
        def _(pe):
            # warm the HAM clock gate during the input load: ~8 matmuls of
            # garbage (psum is overwritten by the first start=True matmul)
            for w in range(8):
                pe.matmul(psB[:, ts(w % 4, NB)], x0[:, 0:128], s0[:, ts(w % 4, NB)],
                          start=True, stop=True)
            for hh in range(HALVES):
                m, h = hh // 2, hh % 2
                ps = pss[hh % 2]
                if hh == 0:
                    pe.wait_ge(k0_sem, 32)  # x0 + s0 resident
                elif hh >= 2:
                    # psum half reuse: ACT of half hh-2 must be done
                    pe.wait_ge(act_sem, hh - 1)
                for ki in range(2):
                    if hh == 0 and ki == 1:
                        pe.wait_ge(k1_sem, 32)  # x1 + s1 resident
                    for nn in range(4):
                        n = 4 * h + nn
                        pe.matmul(
                            ps[:, ts(nn, NB)],
                            xs[ki][:, ts(m, 128)],
                            ss[ki][:, ts(n, NB)],
                            start=(ki == 0),
                            stop=False,
                        )
                if hh == 0:
                    pe.wait_ge(k2_sem, 48)  # x2 + s2 + xq resident
                # norm-row tail: x2/s2 hold 4 partition-replicated copies of
                # the 32 augmented rows, so the 4 banks' K=32 matmuls run
                # CONCURRENTLY in disjoint 32-row groups of the PE array
                for nn in range(4):
                    n = 4 * h + nn
                    mm = pe.matmul(
                        ps[:, ts(nn, NB)],
                        x2[ts(nn, 32), ts(m, 128)],
                        s2[ts(nn, 32), ts(n, NB)],
                        start=False,
                        stop=True,
                        tile_position=(32 * nn, 0),
                    )
                    if nn == 3:
                        mm.then_inc(pe_sem, 1)

        @block.scalar
        def _(act):
            # dummy exp on scratch: hoists the ~2.7us ACT_TABLE_LOAD into the
            # input-load shadow instead of the first real eviction
            act.activation(scratch[:], scratch[:], mybir.ActivationFunctionType.Exp)
            act.wait_ge(k2_sem, 48)  # xq loaded
            for hh in range(HALVES):
                m, h = hh // 2, hh % 2
                if h == 0 and m >= 2:
                    # out row-tile reuse: both half-DMAs of row m-2 done
                    act.wait_ge(od_sem, 16 * (2 * m - 2))
                act.wait_ge(pe_sem, hh + 1)
                act.activation(
                    ots[m % 2][:, ts(h, HALF)],
                    pss[hh % 2][:],
                    mybir.ActivationFunctionType.Exp,
                    bias=xq[:, m : m + 1],
                    scale=2.0 * GAMMA,
                ).then_inc(act_sem, 1)
                # evict to HBM from this engine's own HWDGE ring; the
                # same-engine wait is required — the sequencer dispatches
                # ahead of datapath completion, so without it the DMA
                # reads ot while the ACTIVATE is still writing it
                act.wait_ge(act_sem, hh + 1)
                act.dma_start(
                    out[ts(m, 128), ts(h, HALF)], ots[m % 2][:, ts(h, HALF)]
                ).then_inc(od_sem, 16)
            act.wait_ge(od_sem, 16 * HALVES)

    return nc


_NC_CACHE: bass.Bass | None = None


def _get_nc() -> bass.Bass:
    global _NC_CACHE
    if _NC_CACHE is None:
        _NC_CACHE = _build()
    return _NC_CACHE


def _prepare_in_maps(x: np.ndarray, s: np.ndarray) -> list[dict[str, np.ndarray]]:
    bf16 = ml_dtypes.bfloat16
    x = np.ascontiguousarray(np.asarray(x, dtype=np.float32))
    s = np.ascontiguousarray(np.asarray(s, dtype=np.float32))

    x64 = x.astype(np.float64)
    s64 = s.astype(np.float64)
    x_sq = np.einsum("bd,bd->b", x64, x64)
    s_sq = np.einsum("nd,nd->n", s64, s64)

    sT = np.ascontiguousarray(s.T.astype(bf16))
    h = (-0.5 * s_sq).astype(np.float32)
    hi = h.astype(bf16)
    lo = (h - hi.astype(np.float32)).astype(bf16)
    # tail block (32 rows) replicated 4x along partitions for tile packing
    tail_s = np.zeros((KTAIL, N), dtype=bf16)
    tail_s[0] = hi
    tail_s[1] = lo
    sTail = np.tile(tail_s, (4, 1))          # (128, N)

    in_maps = []
    for c in range(NCORES):
        xc = x[c * B_LOC : (c + 1) * B_LOC]
        xTc = np.ascontiguousarray(xc.T.astype(bf16))
        tail_x = np.zeros((KTAIL, B_LOC), dtype=bf16)
        tail_x[0] = 1
        tail_x[1] = 1
        xTail = np.tile(tail_x, (4, 1))      # (128, B_LOC)
        xsq_c = np.ascontiguousarray(
            (np.log(OUT_SCALE) - GAMMA * x_sq[c * B_LOC : (c + 1) * B_LOC])
            .astype(np.float32)
            .reshape(M_TILES, 128)
            .T
        )
        in_maps.append(
            {"xt": xTc, "st": sT, "xtail": xTail, "stail": sTail, "xsq": xsq_c}
        )
    return in_maps


def run(x: np.ndarray, s: np.ndarray, trace: bool = False, tmpdir: str | None = None):
    """Returns (full (8192, 4096) fp32 output, BassKernelResults)."""
    nc = _get_nc()
    in_maps = _prepare_in_maps(x, s)
    res = run_bass_kernel_spmd(
        nc, in_maps, core_ids=list(range(NCORES)), trace=trace, tmpdir=tmpdir
    )
    full = np.concatenate([np.asarray(r["out"]) for r in res.results], axis=0)
    full = (full.astype(np.float32)) * np.float32(1.0 / OUT_SCALE)
    return full, res


def kernel(**inputs: np.ndarray) -> np.ndarray:
    full, _ = run(inputs["inputs"], inputs["sample_matrix"], trace=False)
    return full


# revision 14
# speedup vs baseline: 1.0233x; 1.0233x over previous
"""RBF similarity: out[b, n] = exp(-gamma * ||inputs[b] - sample_matrix[n]||^2).

Strategy (8 trn2 NeuronCores, data-parallel over query rows):
  - Shard B=8192 query rows into 8 shards of 1024; replicate sample_matrix.
  - GEMM trick: -gamma*||x-s||^2 = 2g*x.s - g*||x||^2 - g*||s||^2.
  - Device computes psum = x_bf16.T @ s_bf16 augmented with 2 extra
    contraction rows that carry -0.5*||s||^2 (split hi/lo in bf16 so the
    norm term keeps ~fp32 precision), then one ScalarE activation per
    4-bank PSUM half evicts it as exp(2g*psum - g*||x||^2), using the
    per-partition bias input for the -g*||x||^2 term.
  - Raw bass (manual semaphores): the walrus build here allows at most
    one sync-wait per instruction, which Tile's scheduler exceeds.
  - Host does layout prep only: transpose to contraction-major, bf16
    cast, and the (negligible-FLOP) row norms.

Pipeline per core: PE fills one 4-bank PSUM half (12 matmuls) while ACT
evicts the other; output rows ping-pong between two SBUF row-tiles whose
DMA-out overlaps the next row's compute.
"""

import numpy as np
import ml_dtypes

import concourse.bass as bass
import concourse.mybir as mybir
from concourse.bass import ts
from concourse.bass_utils import run_bass_kernel_spmd

GAMMA = 0.001
B, D, N = 8192, 256, 4096
NCORES = 8
B_LOC = B // NCORES          # 1024 query rows per core
M_TILES = B_LOC // 128       # 8 PSUM-partition tiles
KTAIL = 32                   # augmented tail k-tile: rows 0/1 carry -0.5*||s||^2 hi/lo
NB = 512                     # matmul free dim = one PSUM bank (fp32)
HALF = 2048                  # 4 banks per PSUM half
HALVES = 2 * M_TILES         # 16 half-iterations

BF16 = mybir.dt.bfloat16
F32 = mybir.dt.float32
U16 = mybir.dt.uint16
OUT_SCALE = 65535.0  # device writes round(out * 65535) as uint16; host rescales


def _build() -> bass.Bass:
    nc = bass.Bass(name="rbf_similarity", trn_type="TRN2")
    xT = nc.dram_tensor("xt", [D, B_LOC], BF16, kind="ExternalInput")
    sT = nc.dram_tensor("st", [D, N], BF16, kind="ExternalInput")
    xTl = nc.dram_tensor("xtail", [128, B_LOC], BF16, kind="ExternalInput")
    sTl = nc.dram_tensor("stail", [128, N], BF16, kind="ExternalInput")
    xsq = nc.dram_tensor("xsq", [128, M_TILES], F32, kind="ExternalInput")
    out = nc.dram_tensor("out", [B_LOC, N], U16, kind="ExternalOutput")

    with (
        nc.sbuf_tensor([128, B_LOC], BF16) as x0,
        nc.sbuf_tensor([128, B_LOC], BF16) as x1,
        nc.sbuf_tensor([128, B_LOC], BF16) as x2,
        nc.sbuf_tensor([128, N], BF16) as s0,
        nc.sbuf_tensor([128, N], BF16) as s1,
        nc.sbuf_tensor([128, N], BF16) as s2,
        nc.sbuf_tensor([128, M_TILES], F32) as xq,
        nc.sbuf_tensor([128, 1], F32) as scratch,
        nc.sbuf_tensor([128, N], U16) as ot0,
        nc.sbuf_tensor([128, N], U16) as ot1,
        nc.psum_tensor([128, HALF], F32) as psA,
        nc.psum_tensor([128, HALF], F32) as psB,
        nc.semaphore("k0_sem") as k0_sem,
        nc.semaphore("k1_sem") as k1_sem,
        nc.semaphore("k2_sem") as k2_sem,
        nc.semaphore("pe_sem") as pe_sem,
        nc.semaphore("act_sem") as act_sem,
        nc.semaphore("od_sem") as od_sem,
        nc.Block() as block,
    ):
        xs = [x0, x1, x2]
        ss = [s0, s1, s2]
        ots = [ot0, ot1]
        pss = [psA, psB]

        @block.sync
        def _(sync):
            # per-k-group semaphores: the PE starts after 1.25 MB (k0 group)
            # instead of the full 4.7 MB input load
            sync.dma_start(x0[:], xT[0:128, :]).then_inc(k0_sem, 16)
            sync.dma_start(s0[:], sT[0:128, :]).then_inc(k0_sem, 16)
            sync.dma_start(x1[:], xT[128:256, :]).then_inc(k1_sem, 16)
            sync.dma_start(s1[:], sT[128:256, :]).then_inc(k1_sem, 16)
            sync.dma_start(x2[:], xTl[:, :]).then_inc(k2_sem, 16)
            sync.dma_start(s2[:], sTl[:, :]).then_inc(k2_sem, 16)
            sync.dma_start(xq[:], xsq[:, :]).then_inc(k2_sem, 16)
            for m in range(M_TILES):
                sync.wait_ge(act_sem, 2 * (m + 1))
                # full 128-row output stripe: contiguous 1 MB in DRAM
                sync.dma_start(out[ts(m, 128), :], ots[m % 2][:]).then_inc(
                    od_sem, 16
                )
            sync.wait_ge(od_sem, 16 * M_TILES)

        @block.tensor
        def _(pe):
            # warm the HAM clock gate during the input load: ~8 matmuls of
            # garbage (psum is overwritten by the first start=True matmul)
            for w in range(8):
                pe.matmul(psB[:, ts(w % 4, NB)], x0[:, 0:128], s0[:, ts(w % 4, NB)],
                          start=True, stop=True)
            for hh in range(HALVES):
                m, h = hh // 2, hh % 2
                ps = pss[hh % 2]
                if hh == 0:
                    pe.wait_ge(k0_sem, 32)  # x0 + s0 resident
                elif hh >= 2:
                    # psum half reuse: ACT of half hh-2 must be done
                    pe.wait_ge(act_sem, hh - 1)
                for ki in range(2):
                    if hh == 0 and ki == 1:
                        pe.wait_ge(k1_sem, 32)  # x1 + s1 resident
                    for nn in range(4):
                        n = 4 * h + nn
                        pe.matmul(
                            ps[:, ts(nn, NB)],
                            xs[ki][:, ts(m, 128)],
                            ss[ki][:, ts(n, NB)],
                            start=(ki == 0),
                            stop=False,
                        )
                if hh == 0:
                    pe.wait_ge(k2_sem, 48)  # x2 + s2 + xq resident
                # norm-row tail: x2/s2 hold 4 partition-replicated copies of
                # the 32 augmented rows, so the 4 banks' K=32 matmuls run
                # CONCURRENTLY in disjoint 32-row groups of the PE array
                for nn in range(4):
                    n = 4 * h + nn
                    mm = pe.matmul(
                        ps[:, ts(nn, NB)],
                        x2[ts(nn, 32), ts(m, 128)],
                        s2[ts(nn, 32), ts(n, NB)],
                        start=False,
                        stop=True,
                        tile_position=(32 * nn, 0),
                    )
                    if nn == 3:
                        mm.then_inc(pe_sem, 1)

        @block.scalar
        def _(act):
            # dummy exp on scratch: hoists the ~2.7us ACT_TABLE_LOAD into the
            # input-load shadow instead of the first real eviction
            act.activation(scratch[:], scratch[:], mybir.ActivationFunctionType.Exp)
            act.wait_ge(k2_sem, 48)  # xq loaded
            for hh in range(HALVES):
                m, h = hh // 2, hh % 2
                if h == 0 and m >= 2:
                    # out row-tile reuse: DMA of row m-2 done
                    act.wait_ge(od_sem, 16 * (m - 1))
                act.wait_ge(pe_sem, hh + 1)
                act.activation(
                    ots[m % 2][:, ts(h, HALF)],
                    pss[hh % 2][:],
                    mybir.ActivationFunctionType.Exp,
                    bias=xq[:, m : m + 1],
                    scale=2.0 * GAMMA,
                ).then_inc(act_sem, 1)

    return nc


_NC_CACHE: bass.Bass | None = None


def _get_nc() -> bass.Bass:
    global _NC_CACHE
    if _NC_CACHE is None:
        _NC_CACHE = _build()
    return _NC_CACHE


def _prepare_in_maps(x: np.ndarray, s: np.ndarray) -> list[dict[str, np.ndarray]]:
    bf16 = ml_dtypes.bfloat16
    x = np.ascontiguousarray(np.asarray(x, dtype=np.float32))
    s = np.ascontiguousarray(np.asarray(s, dtype=np.float32))

    x64 = x.astype(np.float64)
    s64 = s.astype(np.float64)
    x_sq = np.einsum("bd,bd->b", x64, x64)
    s_sq = np.einsum("nd,nd->n", s64, s64)

    sT = np.ascontiguousarray(s.T.astype(bf16))
    h = (-0.5 * s_sq).astype(np.float32)
    hi = h.astype(bf16)
    lo = (h - hi.astype(np.float32)).astype(bf16)
    # tail block (32 rows) replicated 4x along partitions for tile packing
    tail_s = np.zeros((KTAIL, N), dtype=bf16)
    tail_s[0] = hi
    tail_s[1] = lo
    sTail = np.tile(tail_s, (4, 1))          # (128, N)

    in_maps = []
    for c in range(NCORES):
        xc = x[c * B_LOC : (c + 1) * B_LOC]
        xTc = np.ascontiguousarray(xc.T.astype(bf16))
        tail_x = np.zeros((KTAIL, B_LOC), dtype=bf16)
        tail_x[0] = 1
        tail_x[1] = 1
        xTail = np.tile(tail_x, (4, 1))      # (128, B_LOC)
        xsq_c = np.ascontiguousarray(
            (np.log(OUT_SCALE) - GAMMA * x_sq[c * B_LOC : (c + 1) * B_LOC])
            .astype(np.float32)
            .reshape(M_TILES, 128)
            .T
        )
        in_maps.append(
            {"xt": xTc, "st": sT, "xtail": xTail, "stail": sTail, "xsq": xsq_c}
        )
    return in_maps


def run(x: np.ndarray, s: np.ndarray, trace: bool = False, tmpdir: str | None = None):
    """Returns (full (8192, 4096) fp32 output, BassKernelResults)."""
    nc = _get_nc()
    in_maps = _prepare_in_maps(x, s)
    res = run_bass_kernel_spmd(
        nc, in_maps, core_ids=list(range(NCORES)), trace=trace, tmpdir=tmpdir
    )
    full = np.concatenate([np.asarray(r["out"]) for r in res.results], axis=0)
    full = (full.astype(np.float32)) * np.float32(1.0 / OUT_SCALE)
    return full, res


def kernel(**inputs: np.ndarray) -> np.ndarray:
    full, _ = run(inputs["inputs"], inputs["sample_matrix"], trace=False)
    return full


# revision 15
# speedup vs baseline: 1.1082x; 1.0830x over previous
"""RBF similarity: out[b, n] = exp(-gamma * ||inputs[b] - sample_matrix[n]||^2).

Strategy (8 trn2 NeuronCores, data-parallel over query rows):
  - Shard B=8192 query rows into 8 shards of 1024; replicate sample_matrix.
  - GEMM trick: -gamma*||x-s||^2 = 2g*x.s - g*||x||^2 - g*||s||^2.
  - Device computes psum = x_bf16.T @ s_bf16 augmented with 2 extra
    contraction rows that carry -0.5*||s||^2 (split hi/lo in bf16 so the
    norm term keeps ~fp32 precision), then one ScalarE activation per
    4-bank PSUM half evicts it as exp(2g*psum - g*||x||^2), using the
    per-partition bias input for the -g*||x||^2 term.
  - Raw bass (manual semaphores): the walrus build here allows at most
    one sync-wait per instruction, which Tile's scheduler exceeds.
  - Host does layout prep only: transpose to contraction-major, bf16
    cast, and the (negligible-FLOP) row norms.

Pipeline per core: PE fills one 4-bank PSUM half (12 matmuls) while ACT
evicts the other; output rows ping-pong between two SBUF row-tiles whose
DMA-out overlaps the next row's compute.
"""

import numpy as np
import ml_dtypes

import concourse.bass as bass
import concourse.mybir as mybir
from concourse.bass import ts
from concourse.bass_utils import run_bass_kernel_spmd

GAMMA = 0.001
B, D, N = 8192, 256, 4096
NCORES = 8
B_LOC = B // NCORES          # 1024 query rows per core
M_TILES = B_LOC // 128       # 8 PSUM-partition tiles
KTAIL = 32                   # augmented tail k-tile: rows 0/1 carry -0.5*||s||^2 hi/lo
NB = 512                     # matmul free dim = one PSUM bank (fp32)
HALF = 2048                  # 4 banks per PSUM half
HALVES = 2 * M_TILES         # 16 half-iterations

BF16 = mybir.dt.bfloat16
F32 = mybir.dt.float32
U16 = mybir.dt.uint16
OUT_SCALE = 65535.0  # device writes round(out * 65535) as uint16; host rescales


def _build() -> bass.Bass:
    nc = bass.Bass(name="rbf_similarity", trn_type="TRN2")
    xT = nc.dram_tensor("xt", [D, B_LOC], BF16, kind="ExternalInput")
    sT = nc.dram_tensor("st", [D, N], BF16, kind="ExternalInput")
    xTl = nc.dram_tensor("xtail", [128, B_LOC], BF16, kind="ExternalInput")
    sTl = nc.dram_tensor("stail", [128, N], BF16, kind="ExternalInput")
    xsq = nc.dram_tensor("xsq", [128, M_TILES], F32, kind="ExternalInput")
    out = nc.dram_tensor("out", [B_LOC, N], U16, kind="ExternalOutput")

    with (
        nc.sbuf_tensor([128, B_LOC], BF16) as x0,
        nc.sbuf_tensor([128, B_LOC], BF16) as x1,
        nc.sbuf_tensor([128, B_LOC], BF16) as x2,
        nc.sbuf_tensor([128, N], BF16) as s0,
        nc.sbuf_tensor([128, N], BF16) as s1,
        nc.sbuf_tensor([128, N], BF16) as s2,
        nc.sbuf_tensor([128, M_TILES], F32) as xq,
        nc.sbuf_tensor([128, 1], F32) as scratch,
        nc.sbuf_tensor([128, N], U16) as ot0,
        nc.sbuf_tensor([128, N], U16) as ot1,
        nc.sbuf_tensor([128, N], U16) as ot2,
        nc.sbuf_tensor([128, N], U16) as ot3,
        nc.psum_tensor([128, HALF], F32) as psA,
        nc.psum_tensor([128, HALF], F32) as psB,
        nc.semaphore("k0_sem") as k0_sem,
        nc.semaphore("k1_sem") as k1_sem,
        nc.semaphore("k2_sem") as k2_sem,
        nc.semaphore("pe_sem") as pe_sem,
        nc.semaphore("act_sem") as act_sem,
        nc.semaphore("od_sem") as od_sem,
        nc.Block() as block,
    ):
        xs = [x0, x1, x2]
        ss = [s0, s1, s2]
        ots = [ot0, ot1, ot2, ot3]
        pss = [psA, psB]

        @block.sync
        def _(sync):
            # per-k-group semaphores: the PE starts after 1.25 MB (k0 group)
            # instead of the full 4.7 MB input load
            sync.dma_start(x0[:], xT[0:128, :]).then_inc(k0_sem, 16)
            sync.dma_start(s0[:], sT[0:128, :]).then_inc(k0_sem, 16)
            sync.dma_start(x1[:], xT[128:256, :]).then_inc(k1_sem, 16)
            sync.dma_start(s1[:], sT[128:256, :]).then_inc(k1_sem, 16)
            sync.dma_start(x2[:], xTl[:, :]).then_inc(k2_sem, 16)
            sync.dma_start(s2[:], sTl[:, :]).then_inc(k2_sem, 16)
            sync.dma_start(xq[:], xsq[:, :]).then_inc(k2_sem, 16)
            for m in range(M_TILES):
                sync.wait_ge(act_sem, 2 * (m + 1))
                # full 128-row output stripe: contiguous 1 MB in DRAM
                sync.dma_start(out[ts(m, 128), :], ots[m % 4][:]).then_inc(
                    od_sem, 16
                )
            sync.wait_ge(od_sem, 16 * M_TILES)

        @block.tensor
        def _(pe):
            # warm the HAM clock gate during the input load: ~8 matmuls of
            # garbage (psum is overwritten by the first start=True matmul)
            for w in range(8):
                pe.matmul(psB[:, ts(w % 4, NB)], x0[:, 0:128], s0[:, ts(w % 4, NB)],
                          start=True, stop=True)
            for hh in range(HALVES):
                m, h = hh // 2, hh % 2
                ps = pss[hh % 2]
                if hh == 0:
                    pe.wait_ge(k0_sem, 32)  # x0 + s0 resident
                elif hh >= 2:
                    # psum half reuse: ACT of half hh-2 must be done
                    pe.wait_ge(act_sem, hh - 1)
                for ki in range(2):
                    if hh == 0 and ki == 1:
                        pe.wait_ge(k1_sem, 32)  # x1 + s1 resident
                    for nn in range(4):
                        n = 4 * h + nn
                        pe.matmul(
                            ps[:, ts(nn, NB)],
                            xs[ki][:, ts(m, 128)],
                            ss[ki][:, ts(n, NB)],
                            start=(ki == 0),
                            stop=False,
                        )
                if hh == 0:
                    pe.wait_ge(k2_sem, 48)  # x2 + s2 + xq resident
                # norm-row tail: x2/s2 hold 4 partition-replicated copies of
                # the 32 augmented rows, so the 4 banks' K=32 matmuls run
                # CONCURRENTLY in disjoint 32-row groups of the PE array
                for nn in range(4):
                    n = 4 * h + nn
                    mm = pe.matmul(
                        ps[:, ts(nn, NB)],
                        x2[ts(nn, 32), ts(m, 128)],
                        s2[ts(nn, 32), ts(n, NB)],
                        start=False,
                        stop=True,
                        tile_position=(32 * nn, 0),
                    )
                    if nn == 3:
                        mm.then_inc(pe_sem, 1)

        @block.scalar
        def _(act):
            # dummy exp on scratch: hoists the ~2.7us ACT_TABLE_LOAD into the
            # input-load shadow instead of the first real eviction
            act.activation(scratch[:], scratch[:], mybir.ActivationFunctionType.Exp)
            act.wait_ge(k2_sem, 48)  # xq loaded
            for hh in range(HALVES):
                m, h = hh // 2, hh % 2
                if h == 0 and m >= 4:
                    # out row-tile reuse: DMA of row m-4 done
                    act.wait_ge(od_sem, 16 * (m - 3))
                act.wait_ge(pe_sem, hh + 1)
                act.activation(
                    ots[m % 4][:, ts(h, HALF)],
                    pss[hh % 2][:],
                    mybir.ActivationFunctionType.Exp,
                    bias=xq[:, m : m + 1],
                    scale=2.0 * GAMMA,
                ).then_inc(act_sem, 1)

    return nc


_NC_CACHE: bass.Bass | None = None


def _get_nc() -> bass.Bass:
    global _NC_CACHE
    if _NC_CACHE is None:
        _NC_CACHE = _build()
    return _NC_CACHE


def _prepare_in_maps(x: np.ndarray, s: np.ndarray) -> list[dict[str, np.ndarray]]:
    bf16 = ml_dtypes.bfloat16
    x = np.ascontiguousarray(np.asarray(x, dtype=np.float32))
    s = np.ascontiguousarray(np.asarray(s, dtype=np.float32))

    x64 = x.astype(np.float64)
    s64 = s.astype(np.float64)
    x_sq = np.einsum("bd,bd->b", x64, x64)
    s_sq = np.einsum("nd,nd->n", s64, s64)

    sT = np.ascontiguousarray(s.T.astype(bf16))
    h = (-0.5 * s_sq).astype(np.float32)
    hi = h.astype(bf16)
    lo = (h - hi.astype(np.float32)).astype(bf16)
    # tail block (32 rows) replicated 4x along partitions for tile packing
    tail_s = np.zeros((KTAIL, N), dtype=bf16)
    tail_s[0] = hi
    tail_s[1] = lo
    sTail = np.tile(tail_s, (4, 1))          # (128, N)

    in_maps = []
    for c in range(NCORES):
        xc = x[c * B_LOC : (c + 1) * B_LOC]
        xTc = np.ascontiguousarray(xc.T.astype(bf16))
        tail_x = np.zeros((KTAIL, B_LOC), dtype=bf16)
        tail_x[0] = 1
        tail_x[1] = 1
        xTail = np.tile(tail_x, (4, 1))      # (128, B_LOC)
        xsq_c = np.ascontiguousarray(
            (np.log(OUT_SCALE) - GAMMA * x_sq[c * B_LOC : (c + 1) * B_LOC])
            .astype(np.float32)
            .reshape(M_TILES, 128)
            .T
        )
        in_maps.append(
            {"xt": xTc, "st": sT, "xtail": xTail, "stail": sTail, "xsq": xsq_c}
        )
    return in_maps


def run(x: np.ndarray, s: np.ndarray, trace: bool = False, tmpdir: str | None = None):
    """Returns (full (8192, 4096) fp32 output, BassKernelResults)."""
    nc = _get_nc()
    in_maps = _prepare_in_maps(x, s)
    res = run_bass_kernel_spmd(
        nc, in_maps, core_ids=list(range(NCORES)), trace=trace, tmpdir=tmpdir
    )
    full = np.concatenate([np.asarray(r["out"]) for r in res.results], axis=0)
    full = (full.astype(np.float32)) * np.float32(1.0 / OUT_SCALE)
    return full, res


def kernel(**inputs: np.ndarray) -> np.ndarray:
    full, _ = run(inputs["inputs"], inputs["sample_matrix"], trace=False)
    return full
